# revision 36
# baseline (speedup 1.0000x reference)
"""AGRU (attention-gated GRU) Trainium2 kernel.

Problem: B=2048, T=200, D=U=64, f32.
    pre_r = x @ w_ir + b_ir + b_hr
    pre_h = x @ w_ih + b_ih
    per t: r = sigmoid(pre_r_t + h @ w_hr)
           hc = tanh(pre_h_t + r * (h @ w_hh + b_hh))
           h  = (1-a_t) * h + a_t * hc      (output hs[:, t] = h)

Strategy: pure batch data-parallel over 8 cores (256 batch rows/core).
On-chip layout is "stacked transposed": tiles are [128 partitions, 128 cols]
where partition p = (g, u) with g = p//64 selecting a 128-row batch half and
u = p%64 the unit; column n is the batch index within the half
(b = g*128 + n). All matmuls keep weights as lhsT (blockdiag [128,128]) and
batch on the free axis, so the sequential scan never transposes anything.

The recurrence is numerically chaotic: any per-step quantization noise
(bf16 anywhere) is amplified until errors saturate at O(1). Everything is
fp32.

Per-step engine schedule (SPLIT=True):
  PE : pre_r/pre_h x-projections batched 4 steps per FD=512 matmul into a
       PSUM bank; recurrent Whr/Whh matmuls accumulate into 128-col slices
       of those banks (u/v split: W.h_new = W.u + W.v, so the gate combine
       never sits on the recurrent critical path).
  ACT: r = sigmoid(psum + bias), hc = tanh(psum + bias)  (bias folded free)
  DVE: t1 = (pg + b_hh)*r [scalar_tensor_tensor], ph += t1 (in-place PSUM),
       u = a*hc, h = u + v
  GPSIMD: broadcast of att rows across partitions (compact DMA), and
       v = (1-a)*h (off the critical path).
"""

import sys
import types
import numpy as np

sys.path.insert(0, "/opt/trn_rl_repo")

# Provide the antenv.axon_hooks registry if the image's antenv stub lacks it,
# and register the ctypes NTFF profile hook (trace=True timing path only).
try:
    import antenv.axon_hooks  # noqa: F401
except ImportError:
    _hooks = types.ModuleType("antenv.axon_hooks")
    _hooks._hook = None

    def _set_hook(h):
        _hooks._hook = h

    def _get_hook():
        return _hooks._hook

    _hooks.set_axon_ntff_profile_hook = _set_hook
    _hooks.get_axon_ntff_profile_hook = _get_hook
    sys.modules["antenv.axon_hooks"] = _hooks
    try:
        from trn_agent_boot.trn_boot import _ntff_profile_via_ctypes

        _h = _ntff_profile_via_ctypes("/opt/axon/libaxon_pjrt.so")
        if _h is not None:
            _set_hook(_h)
    except Exception:
        pass

B, T, D, U = 2048, 200, 64, 64
NCORES = 8
BC = B // NCORES          # 256 batch rows per core
NH = 2                    # batch halves stacked on partitions
NB = BC // NH             # 128 batch columns per tile
TC = 20                   # timesteps per DMA chunk
NCHUNK = T // TC
XB = 4                    # timesteps per batched x-projection matmul (FD=512)

SPLIT = True              # u/v split of the recurrent state

_BUILT = {}


def _build_nc(split=SPLIT):
    """Build the Bass graph (single core program, run SPMD on 8 cores)."""
    import concourse.mybir as mybir
    import concourse.tile as tile
    from concourse import bacc
    from contextlib import ExitStack

    F32 = mybir.dt.float32
    AF = mybir.ActivationFunctionType
    OP = mybir.AluOpType

    nc = bacc.Bacc(trn_type="TRN2")

    xT = nc.dram_tensor("xT", [T, NH, D, NB], F32, kind="ExternalInput")
    attA = nc.dram_tensor("attA", [T, NH, U, NB], F32, kind="ExternalInput")
    attM = nc.dram_tensor("attM", [T, NH, U, NB], F32, kind="ExternalInput")
    h0T = nc.dram_tensor("h0T", [NH, U, NB], F32, kind="ExternalInput")
    wk = nc.dram_tensor("wk", [128, 4 * 128], F32, kind="ExternalInput")
    biasr = nc.dram_tensor("biasr", [128, 1], F32, kind="ExternalInput")
    biash = nc.dram_tensor("biash", [128, 1], F32, kind="ExternalInput")
    biasg = nc.dram_tensor("biasg", [128, 1], F32, kind="ExternalInput")
    out = nc.dram_tensor("out", [T, NH, U, NB], F32, kind="ExternalOutput")

    with tile.TileContext(nc) as tc, ExitStack() as ctx:
        const = ctx.enter_context(tc.tile_pool(name="const", bufs=1))
        xpool = ctx.enter_context(tc.tile_pool(name="xp", bufs=2))
        apool = ctx.enter_context(tc.tile_pool(name="apo", bufs=2))
        opool = ctx.enter_context(tc.tile_pool(name="opo", bufs=2))
        tpool = ctx.enter_context(tc.tile_pool(name="tp", bufs=3))
        pbig = ctx.enter_context(tc.tile_pool(name="pb", bufs=2, space="PSUM"))
        ppg = ctx.enter_context(tc.tile_pool(name="pg", bufs=3, space="PSUM"))

        # Constants: weights [128, 4*128] (wir2 | whr2 | wih2 | whh2), biases.
        w_sb = const.tile([128, 4 * 128], F32)
        nc.sync.dma_start(out=w_sb, in_=wk[:, :])
        biasr_sb = const.tile([128, 1], F32)
        nc.sync.dma_start(out=biasr_sb, in_=biasr[:, :])
        biash_sb = const.tile([128, 1], F32)
        nc.sync.dma_start(out=biash_sb, in_=biash[:, :])
        biasg_sb = const.tile([128, 1], F32)
        nc.sync.dma_start(out=biasg_sb, in_=biasg[:, :])
        h0_sb = const.tile([128, NB], F32)
        nc.sync.dma_start(out=h0_sb, in_=h0T.rearrange("g u n -> (g u) n"))
        zero_sb = const.tile([128, NB], F32)
        nc.vector.memset(zero_sb, 0.0)

        wir2 = w_sb[:, 0 * 128:1 * 128]
        whr2 = w_sb[:, 1 * 128:2 * 128]
        wih2 = w_sb[:, 2 * 128:3 * 128]
        whh2 = w_sb[:, 3 * 128:4 * 128]

        u_prev = h0_sb      # u_{t-1}; bootstrap: h0 + 0 split
        v_prev = zero_sb
        h_prev = h0_sb      # h_t as materialized tensor (for v_t)

        chunk_sizes = [TC] * NCHUNK
        t_off = 0
        for c, TCc in enumerate(chunk_sizes):
            ts = slice(t_off, t_off + TCc)
            t_off += TCc
            npiece = 4 if c == 0 else 1
            pc = TCc // npiece
            x_sb = xpool.tile([128, TCc * NB], F32)
            a_sb = apool.tile([128, TCc * NB], F32, tag="a")
            m_sb = apool.tile([128, TCc * NB], F32, tag="m")
            for p in range(npiece):
                pt = slice(ts.start + p * pc, ts.start + (p + 1) * pc)
                pf = slice(p * pc * NB, (p + 1) * pc * NB)
                nc.sync.dma_start(
                    out=x_sb[:, pf].rearrange("p (t n) -> p t n", t=pc),
                    in_=xT[pt].rearrange("t g d n -> (g d) t n"))
                nc.sync.dma_start(
                    out=a_sb[:, pf].rearrange("p (t n) -> p t n", t=pc),
                    in_=attA[pt].rearrange("t g u n -> (g u) t n"))
                nc.sync.dma_start(
                    out=m_sb[:, pf].rearrange("p (t n) -> p t n", t=pc),
                    in_=attM[pt].rearrange("t g u n -> (g u) t n"))
            o_sb = opool.tile([128, TCc * NB], F32)

            if True:
                for k in range(TCc):
                    j = k
                    s = slice(j * NB, (j + 1) * NB)
                    pr = pbig.tile([128, NB], F32, tag="prx")
                    ph = pbig.tile([128, NB], F32, tag="phx")
                    pg = ppg.tile([128, NB], F32, tag="pg")

                    # v_t early: fills the DVE idle slot at step start and
                    # unblocks mm_rv/mm_gv long before sigma needs pr.
                    v_t = tpool.tile([128, NB], F32, tag="v")
                    nc.vector.tensor_mul(v_t, m_sb[:, s], h_prev)

                    nc.tensor.matmul(pr, wir2, x_sb[:, s], start=True,
                                     stop=False)
                    nc.tensor.matmul(ph, wih2, x_sb[:, s], start=True,
                                     stop=True)
                    if split:
                        nc.tensor.matmul(pr, whr2, v_prev, start=False,
                                         stop=False, skip_group_check=True)
                        nc.tensor.matmul(pg, whh2, v_prev, start=True,
                                         stop=False)
                        nc.tensor.matmul(pr, whr2, u_prev, start=False,
                                         stop=True, skip_group_check=True)
                        nc.tensor.matmul(pg, whh2, u_prev, start=False,
                                         stop=True)
                    else:
                        nc.tensor.matmul(pr, whr2, h_prev, start=False,
                                         stop=True, skip_group_check=True)
                        nc.tensor.matmul(pg, whh2, h_prev, start=True,
                                         stop=True)

                    # r = sigmoid(pr + (b_ir + b_hr))
                    r = tpool.tile([128, NB], F32, tag="r")
                    nc.scalar.activation(r, pr, AF.Sigmoid, bias=biasr_sb,
                                         scale=1.0)
                    # t1 = (pg + b_hh) * r ; ph += t1 ; hc = tanh(ph+b_ih)
                    t1 = tpool.tile([128, NB], F32, tag="t1")
                    nc.vector.scalar_tensor_tensor(t1, pg, biasg_sb, r,
                                                   OP.add, OP.mult)
                    nc.vector.tensor_add(ph, t1, ph)
                    hc = tpool.tile([128, NB], F32, tag="hc")
                    nc.scalar.activation(hc, ph, AF.Tanh, bias=biash_sb,
                                         scale=1.0)
                    # u = a*hc ; h_new = u+v
                    u_t = tpool.tile([128, NB], F32, tag="u")
                    nc.vector.tensor_mul(u_t, a_sb[:, s], hc)
                    nc.vector.tensor_add(o_sb[:, s], u_t, v_t)

                    u_prev, v_prev = u_t, v_t
                    h_prev = o_sb[:, s]

            nc.sync.dma_start(
                out=out[ts].rearrange("t g u n -> (g u) t n"),
                in_=o_sb.rearrange("p (t n) -> p t n", t=TCc),
            )
    nc.compile()
    return nc


def _get_nc():
    if "nc" not in _BUILT:
        _BUILT["nc"] = _build_nc()
    return _BUILT["nc"]


def _prep_inputs(x, att, h0, w_ir, w_hr, b_ir, b_hr, w_ih, w_hh, b_ih, b_hh):
    """Host-side layout prep. Returns per-core in_maps list."""
    x = np.ascontiguousarray(np.asarray(x, np.float32))
    att = np.ascontiguousarray(np.asarray(att, np.float32).reshape(B, T))
    h0 = np.asarray(h0, np.float32)

    def blk(w):
        z = np.zeros((128, 128), np.float32)
        z[:64, :64] = w
        z[64:, 64:] = w
        return z

    wk = np.stack([blk(np.asarray(w_ir, np.float32)),
                   blk(np.asarray(w_hr, np.float32)),
                   blk(np.asarray(w_ih, np.float32)),
                   blk(np.asarray(w_hh, np.float32))])
    wk = np.ascontiguousarray(wk.transpose(1, 0, 2).reshape(128, 512))

    br = np.tile(np.asarray(b_ir, np.float32) + np.asarray(b_hr, np.float32), 2)
    bh = np.tile(np.asarray(b_ih, np.float32), 2)
    bg = np.tile(np.asarray(b_hh, np.float32), 2)
    biasr = np.ascontiguousarray(br.reshape(128, 1))
    biash = np.ascontiguousarray(bh.reshape(128, 1))
    biasg = np.ascontiguousarray(bg.reshape(128, 1))

    # x: [B, T, D] -> per core [T, NH, D, NB]
    xc = x.reshape(NCORES, NH, NB, T, D)
    xTn = np.ascontiguousarray(xc.transpose(0, 3, 1, 4, 2))

    # att: [B, T] -> per core [T, NH, U, NB] (host-amplified over U)
    ac = att.reshape(NCORES, NH, NB, T).transpose(0, 3, 1, 2)  # [core,T,NH,NB]
    attAc = np.ascontiguousarray(
        np.broadcast_to(ac[:, :, :, None, :], (NCORES, T, NH, U, NB)))
    attMc = np.ascontiguousarray(
        np.broadcast_to((1.0 - ac)[:, :, :, None, :], (NCORES, T, NH, U, NB)))

    # h0: [B, U] -> per core [NH, U, NB]
    hc0 = h0.reshape(NCORES, NH, NB, U)
    h0Tn = np.ascontiguousarray(hc0.transpose(0, 1, 3, 2))

    in_maps = []
    for i in range(NCORES):
        in_maps.append({
            "xT": xTn[i], "attA": attAc[i], "attM": attMc[i], "h0T": h0Tn[i],
            "wk": wk, "biasr": biasr, "biash": biash, "biasg": biasg,
        })
    return in_maps


def _postprocess(outs):
    hs = np.stack([np.asarray(o["out"]) for o in outs])   # [8, T, NH, U, NB]
    hs = hs.astype(np.float32).transpose(0, 2, 4, 1, 3)   # [8, NH, NB, T, U]
    return np.ascontiguousarray(hs.reshape(B, T, U))


def _run(inputs, trace=False):
    from concourse.bass_utils import run_bass_kernel_spmd

    in_maps = _prep_inputs(
        inputs["x"], inputs["att_scores"], inputs["h0"],
        inputs["w_ir"], inputs["w_hr"], inputs["b_ir"], inputs["b_hr"],
        inputs["w_ih"], inputs["w_hh"], inputs["b_ih"], inputs["b_hh"],
    )
    nc = _get_nc()
    res = run_bass_kernel_spmd(nc, in_maps, core_ids=list(range(NCORES)),
                               trace=trace)
    return _postprocess(res.results), res


def kernel(**inputs) -> np.ndarray:
    out, _ = _run(inputs, trace=False)
    return out


# revision 38
# speedup vs baseline: 1.1886x; 1.1886x over previous
"""AGRU (attention-gated GRU) Trainium2 kernel.

Problem: B=2048, T=200, D=U=64, f32.
    pre_r = x @ w_ir + b_ir + b_hr
    pre_h = x @ w_ih + b_ih
    per t: r = sigmoid(pre_r_t + h @ w_hr)
           hc = tanh(pre_h_t + r * (h @ w_hh + b_hh))
           h  = (1-a_t) * h + a_t * hc      (output hs[:, t] = h)

Strategy: pure batch data-parallel over 8 cores (256 batch rows/core).
On-chip layout is "stacked transposed": tiles are [128 partitions, 128 cols]
where partition p = (g, u) with g = p//64 selecting a 128-row batch half and
u = p%64 the unit; column n is the batch index within the half
(b = g*128 + n). All matmuls keep weights as lhsT (blockdiag [128,128]) and
batch on the free axis, so the sequential scan never transposes anything.

The recurrence is numerically chaotic: any per-step quantization noise
(bf16 anywhere) is amplified until errors saturate at O(1). Everything is
fp32.

Per-step engine schedule (SPLIT=True):
  PE : 6 fp32 matmuls into PSUM: pre_r = Wir.x_t (+ Whr.u + Whr.v
       accumulated), pre_h = Wih.x_t, pg = Whh.u + Whh.v. The u/v split
       (W.h_new = W.u + W.v, PSUM-accumulated) keeps the h_new = u+v gate
       combine OFF the recurrent critical path.
  ACT: r = sigmoid(psum + bias), hc = tanh(psum + bias)  (bias folded free)
  DVE: v = (1-a)*h first (fills the idle slot, unblocks the v-matmuls),
       t1 = (pg + b_hh)*r [scalar_tensor_tensor], ph += t1 (in-place PSUM),
       u = a*hc, h_new = u + v.

The measured critical cycle per step (~2.5us) is the serial dependency
chain u -> mm_ru -> sigmoid -> t1 -> t2 -> tanh -> u, dominated by the
ScalarE fixed cost (~(N+352)/1.2 ns per activation) and cross-engine
semaphore latency. att tiles are host-amplified over the unit axis
(GPSIMD partition_broadcast can't write at a partition offset, and bulk
GPSIMD work stalls DVE via the shared SBUF port).
"""

import sys
import types
import numpy as np

sys.path.insert(0, "/opt/trn_rl_repo")

# Provide the antenv.axon_hooks registry if the image's antenv stub lacks it,
# and register the ctypes NTFF profile hook (trace=True timing path only).
try:
    import antenv.axon_hooks  # noqa: F401
except ImportError:
    _hooks = types.ModuleType("antenv.axon_hooks")
    _hooks._hook = None

    def _set_hook(h):
        _hooks._hook = h

    def _get_hook():
        return _hooks._hook

    _hooks.set_axon_ntff_profile_hook = _set_hook
    _hooks.get_axon_ntff_profile_hook = _get_hook
    sys.modules["antenv.axon_hooks"] = _hooks
    try:
        from trn_agent_boot.trn_boot import _ntff_profile_via_ctypes

        _h = _ntff_profile_via_ctypes("/opt/axon/libaxon_pjrt.so")
        if _h is not None:
            _set_hook(_h)
    except Exception:
        pass

B, T, D, U = 2048, 200, 64, 64
NCORES = 8
BC = B // NCORES          # 256 batch rows per core
NH = 2                    # batch halves stacked on partitions
NB = BC // NH             # 128 batch columns per tile
TC = 20                   # timesteps per DMA chunk
NCHUNK = T // TC

SPLIT = True              # u/v split of the recurrent state

_BUILT = {}


def _build_nc(split=SPLIT):
    """Build the Bass graph (single core program, run SPMD on 8 cores)."""
    import concourse.mybir as mybir
    import concourse.tile as tile
    from concourse import bacc
    from contextlib import ExitStack

    F32 = mybir.dt.float32
    AF = mybir.ActivationFunctionType
    OP = mybir.AluOpType

    nc = bacc.Bacc(trn_type="TRN2")

    xT = nc.dram_tensor("xT", [T, NH, D, NB], F32, kind="ExternalInput")
    attA = nc.dram_tensor("attA", [T, NH, U, NB], F32, kind="ExternalInput")
    attM = nc.dram_tensor("attM", [T, NH, U, NB], F32, kind="ExternalInput")
    h0T = nc.dram_tensor("h0T", [NH, U, NB], F32, kind="ExternalInput")
    wk = nc.dram_tensor("wk", [128, 4 * 128], F32, kind="ExternalInput")
    biasr = nc.dram_tensor("biasr", [128, 1], F32, kind="ExternalInput")
    biash = nc.dram_tensor("biash", [128, 1], F32, kind="ExternalInput")
    biasg = nc.dram_tensor("biasg", [128, 1], F32, kind="ExternalInput")
    out = nc.dram_tensor("out", [T, NH, U, NB], F32, kind="ExternalOutput")

    with tile.TileContext(nc) as tc, ExitStack() as ctx:
        const = ctx.enter_context(tc.tile_pool(name="const", bufs=1))
        xpool = ctx.enter_context(tc.tile_pool(name="xp", bufs=2))
        apool = ctx.enter_context(tc.tile_pool(name="apo", bufs=2))
        opool = ctx.enter_context(tc.tile_pool(name="opo", bufs=2))
        tpool = ctx.enter_context(tc.tile_pool(name="tp", bufs=3))
        pbig = ctx.enter_context(tc.tile_pool(name="pb", bufs=2, space="PSUM"))
        ppg = ctx.enter_context(tc.tile_pool(name="pg", bufs=3, space="PSUM"))

        # Constants: weights [128, 4*128] (wir2 | whr2 | wih2 | whh2), biases.
        w_sb = const.tile([128, 4 * 128], F32)
        nc.sync.dma_start(out=w_sb, in_=wk[:, :])
        biasr_sb = const.tile([128, 1], F32)
        nc.sync.dma_start(out=biasr_sb, in_=biasr[:, :])
        biash_sb = const.tile([128, 1], F32)
        nc.sync.dma_start(out=biash_sb, in_=biash[:, :])
        biasg_sb = const.tile([128, 1], F32)
        nc.sync.dma_start(out=biasg_sb, in_=biasg[:, :])
        h0_sb = const.tile([128, NB], F32)
        nc.sync.dma_start(out=h0_sb, in_=h0T.rearrange("g u n -> (g u) n"))
        zero_sb = const.tile([128, NB], F32)
        nc.vector.memset(zero_sb, 0.0)

        wir2 = w_sb[:, 0 * 128:1 * 128]
        whr2 = w_sb[:, 1 * 128:2 * 128]
        wih2 = w_sb[:, 2 * 128:3 * 128]
        whh2 = w_sb[:, 3 * 128:4 * 128]

        u_prev = h0_sb      # u_{t-1}; bootstrap: h0 + 0 split
        v_prev = zero_sb
        h_prev = h0_sb      # h_t as materialized tensor (for v_t)

        chunk_sizes = [TC] * NCHUNK
        t_off = 0
        for c, TCc in enumerate(chunk_sizes):
            ts = slice(t_off, t_off + TCc)
            t_off += TCc
            npiece = 4 if c == 0 else 1
            pc = TCc // npiece
            x_sb = xpool.tile([128, TCc * NB], F32)
            a_sb = apool.tile([128, TCc * NB], F32, tag="a")
            m_sb = apool.tile([128, TCc * NB], F32, tag="m")
            for p in range(npiece):
                pt = slice(ts.start + p * pc, ts.start + (p + 1) * pc)
                pf = slice(p * pc * NB, (p + 1) * pc * NB)
                nc.sync.dma_start(
                    out=x_sb[:, pf].rearrange("p (t n) -> p t n", t=pc),
                    in_=xT[pt].rearrange("t g d n -> (g d) t n"))
                nc.sync.dma_start(
                    out=a_sb[:, pf].rearrange("p (t n) -> p t n", t=pc),
                    in_=attA[pt].rearrange("t g u n -> (g u) t n"))
                nc.sync.dma_start(
                    out=m_sb[:, pf].rearrange("p (t n) -> p t n", t=pc),
                    in_=attM[pt].rearrange("t g u n -> (g u) t n"))
            o_sb = opool.tile([128, TCc * NB], F32)

            for bb in range(TCc // 2):
                sbb = slice(bb * 2 * NB, (bb + 1) * 2 * NB)
                prx = pbig.tile([128, 2 * NB], F32, tag="prx")
                phx = pbig.tile([128, 2 * NB], F32, tag="phx")
                nc.tensor.matmul(prx, wir2, x_sb[:, sbb], start=True,
                                 stop=False, skip_group_check=True)
                nc.tensor.matmul(phx, wih2, x_sb[:, sbb], start=True,
                                 stop=True, skip_group_check=True)
                for k in range(2):
                    j = bb * 2 + k
                    s = slice(j * NB, (j + 1) * NB)
                    sk = slice(k * NB, (k + 1) * NB)
                    pr = prx[:, sk]
                    ph = phx[:, sk]
                    pg = ppg.tile([128, NB], F32, tag="pg")

                    # v_t early: fills the DVE idle slot at step start and
                    # unblocks mm_rv/mm_gv long before sigma needs pr.
                    v_t = tpool.tile([128, NB], F32, tag="v")
                    nc.vector.tensor_mul(v_t, m_sb[:, s], h_prev)

                    if split:
                        nc.tensor.matmul(pr, whr2, v_prev, start=False,
                                         stop=False, skip_group_check=True)
                        nc.tensor.matmul(pg, whh2, v_prev, start=True,
                                         stop=False)
                        nc.tensor.matmul(pr, whr2, u_prev, start=False,
                                         stop=True, skip_group_check=True)
                        nc.tensor.matmul(pg, whh2, u_prev, start=False,
                                         stop=True)
                    else:
                        nc.tensor.matmul(pr, whr2, h_prev, start=False,
                                         stop=True, skip_group_check=True)
                        nc.tensor.matmul(pg, whh2, h_prev, start=True,
                                         stop=True)

                    # r = sigmoid(pr + (b_ir + b_hr))
                    r = tpool.tile([128, NB], F32, tag="r")
                    nc.scalar.activation(r, pr, AF.Sigmoid, bias=biasr_sb,
                                         scale=1.0)
                    # t1 = (pg + b_hh) * r ; ph += t1 ; hc = tanh(ph+b_ih)
                    t1 = tpool.tile([128, NB], F32, tag="t1")
                    nc.vector.scalar_tensor_tensor(t1, pg, biasg_sb, r,
                                                   OP.add, OP.mult)
                    nc.vector.tensor_add(ph, t1, ph)
                    hc = tpool.tile([128, NB], F32, tag="hc")
                    nc.scalar.activation(hc, ph, AF.Tanh, bias=biash_sb,
                                         scale=1.0)
                    # u = a*hc ; h_new = u+v
                    u_t = tpool.tile([128, NB], F32, tag="u")
                    nc.vector.tensor_mul(u_t, a_sb[:, s], hc)
                    nc.vector.tensor_add(o_sb[:, s], u_t, v_t)

                    u_prev, v_prev = u_t, v_t
                    h_prev = o_sb[:, s]

            nc.sync.dma_start(
                out=out[ts].rearrange("t g u n -> (g u) t n"),
                in_=o_sb.rearrange("p (t n) -> p t n", t=TCc),
            )
    nc.compile()
    return nc


def _get_nc():
    if "nc" not in _BUILT:
        _BUILT["nc"] = _build_nc()
    return _BUILT["nc"]


def _prep_inputs(x, att, h0, w_ir, w_hr, b_ir, b_hr, w_ih, w_hh, b_ih, b_hh):
    """Host-side layout prep. Returns per-core in_maps list."""
    x = np.ascontiguousarray(np.asarray(x, np.float32))
    att = np.ascontiguousarray(np.asarray(att, np.float32).reshape(B, T))
    h0 = np.asarray(h0, np.float32)

    def blk(w):
        z = np.zeros((128, 128), np.float32)
        z[:64, :64] = w
        z[64:, 64:] = w
        return z

    wk = np.stack([blk(np.asarray(w_ir, np.float32)),
                   blk(np.asarray(w_hr, np.float32)),
                   blk(np.asarray(w_ih, np.float32)),
                   blk(np.asarray(w_hh, np.float32))])
    wk = np.ascontiguousarray(wk.transpose(1, 0, 2).reshape(128, 512))

    br = np.tile(np.asarray(b_ir, np.float32) + np.asarray(b_hr, np.float32), 2)
    bh = np.tile(np.asarray(b_ih, np.float32), 2)
    bg = np.tile(np.asarray(b_hh, np.float32), 2)
    biasr = np.ascontiguousarray(br.reshape(128, 1))
    biash = np.ascontiguousarray(bh.reshape(128, 1))
    biasg = np.ascontiguousarray(bg.reshape(128, 1))

    # x: [B, T, D] -> per core [T, NH, D, NB]
    xc = x.reshape(NCORES, NH, NB, T, D)
    xTn = np.ascontiguousarray(xc.transpose(0, 3, 1, 4, 2))

    # att: [B, T] -> per core [T, NH, U, NB] (host-amplified over U)
    ac = att.reshape(NCORES, NH, NB, T).transpose(0, 3, 1, 2)  # [core,T,NH,NB]
    attAc = np.ascontiguousarray(
        np.broadcast_to(ac[:, :, :, None, :], (NCORES, T, NH, U, NB)))
    attMc = np.ascontiguousarray(
        np.broadcast_to((1.0 - ac)[:, :, :, None, :], (NCORES, T, NH, U, NB)))

    # h0: [B, U] -> per core [NH, U, NB]
    hc0 = h0.reshape(NCORES, NH, NB, U)
    h0Tn = np.ascontiguousarray(hc0.transpose(0, 1, 3, 2))

    in_maps = []
    for i in range(NCORES):
        in_maps.append({
            "xT": xTn[i], "attA": attAc[i], "attM": attMc[i], "h0T": h0Tn[i],
            "wk": wk, "biasr": biasr, "biash": biash, "biasg": biasg,
        })
    return in_maps


def _postprocess(outs):
    hs = np.stack([np.asarray(o["out"]) for o in outs])   # [8, T, NH, U, NB]
    hs = hs.astype(np.float32).transpose(0, 2, 4, 1, 3)   # [8, NH, NB, T, U]
    return np.ascontiguousarray(hs.reshape(B, T, U))


def _run(inputs, trace=False):
    from concourse.bass_utils import run_bass_kernel_spmd

    in_maps = _prep_inputs(
        inputs["x"], inputs["att_scores"], inputs["h0"],
        inputs["w_ir"], inputs["w_hr"], inputs["b_ir"], inputs["b_hr"],
        inputs["w_ih"], inputs["w_hh"], inputs["b_ih"], inputs["b_hh"],
    )
    nc = _get_nc()
    res = run_bass_kernel_spmd(nc, in_maps, core_ids=list(range(NCORES)),
                               trace=trace)
    return _postprocess(res.results), res


def kernel(**inputs) -> np.ndarray:
    out, _ = _run(inputs, trace=False)
    return out
